# revision 1
# baseline (speedup 1.0000x reference)
import numpy as np

B, C, H_IMG, W_IMG = 32, 192, 56, 56
NUM_HEADS, AGENT_NUM, POOL = 6, 49, 7
N_CORES = 8
N = H_IMG * W_IMG
HD = C // NUM_HEADS
SCALE = HD ** (-0.5)


def _interp_matrix(out_size: int, in_size: int) -> np.ndarray:
    """Bilinear (half-pixel-center, edge-clamped) interpolation matrix.

    Matches jax.image.resize(method="linear") for upsampling: row o gives the
    weights over input cells for output cell o.
    """
    m = np.zeros((out_size, in_size), dtype=np.float64)
    ratio = in_size / out_size
    for o in range(out_size):
        s = (o + 0.5) * ratio - 0.5
        i0 = int(np.floor(s))
        frac = s - i0
        lo = min(max(i0, 0), in_size - 1)
        hi = min(max(i0 + 1, 0), in_size - 1)
        m[o, lo] += 1.0 - frac
        m[o, hi] += frac
    return m.astype(np.float32)


def _np_pos_biases(an_bias, na_bias, ah_bias, aw_bias, ha_bias, wa_bias):
    mh = _interp_matrix(H_IMG, POOL)
    mw = _interp_matrix(W_IMG, POOL)
    pb1 = np.einsum("Hj,hajk,Wk->haHW", mh, an_bias, mw).reshape(NUM_HEADS, AGENT_NUM, N)
    pos_bias = (pb1[None] + (ah_bias + aw_bias).reshape(1, NUM_HEADS, AGENT_NUM, N))
    ab1 = np.einsum("Hj,hajk,Wk->haHW", mh, na_bias, mw).reshape(NUM_HEADS, AGENT_NUM, N)
    agent_bias = (ab1[None].transpose(0, 1, 3, 2)
                  + (ha_bias + wa_bias).reshape(1, NUM_HEADS, N, AGENT_NUM))
    return pos_bias.astype(np.float32), agent_bias.astype(np.float32)


def _forward_np(x, Wqkv, bqkv, proj_w, proj_b, dwc_w, dwc_b,
                pos_bias, agent_bias):
    b = x.shape[0]
    c, n, nh, A, hd = C, N, NUM_HEADS, AGENT_NUM, HD

    xf = x.reshape(b, c, n).transpose(0, 2, 1)
    qkv = xf @ Wqkv + bqkv
    q, k, v = qkv[..., :c], qkv[..., c:2 * c], qkv[..., 2 * c:]

    qi = q.reshape(b, POOL, H_IMG // POOL, POOL, W_IMG // POOL, c)
    agent = qi.mean(axis=(2, 4)).reshape(b, A, c)

    qh = q.reshape(b, n, nh, hd).transpose(0, 2, 1, 3)
    kh = k.reshape(b, n, nh, hd).transpose(0, 2, 1, 3)
    vh = v.reshape(b, n, nh, hd).transpose(0, 2, 1, 3)
    ah = agent.reshape(b, A, nh, hd).transpose(0, 2, 1, 3)

    s1 = np.einsum("bhad,bhnd->bhan", ah * SCALE, kh) + pos_bias
    s1 = s1 - s1.max(axis=-1, keepdims=True)
    e1 = np.exp(s1)
    attn1 = e1 / e1.sum(axis=-1, keepdims=True)
    agent_v = np.einsum("bhan,bhnd->bhad", attn1, vh)

    s2 = np.einsum("bhnd,bhad->bhna", qh * SCALE, ah) + agent_bias
    s2 = s2 - s2.max(axis=-1, keepdims=True)
    e2 = np.exp(s2)
    attn2 = e2 / e2.sum(axis=-1, keepdims=True)
    out = np.einsum("bhna,bhad->bhnd", attn2, agent_v)
    out = out.transpose(0, 2, 1, 3).reshape(b, n, c)

    vimg = vh.transpose(0, 2, 1, 3).reshape(b, H_IMG, W_IMG, c).transpose(0, 3, 1, 2)
    vp = np.pad(vimg, ((0, 0), (0, 0), (1, 1), (1, 1)))
    dw = np.zeros_like(vimg)
    for di in range(3):
        for dj in range(3):
            dw += dwc_w[None, :, 0, di, dj, None, None] * \
                vp[:, :, di:di + H_IMG, dj:dj + W_IMG]
    dw = dw + dwc_b[None, :, None, None]
    out = out + dw.transpose(0, 2, 3, 1).reshape(b, n, c)

    out = out @ proj_w + proj_b
    return out.transpose(0, 2, 1).reshape(b, c, H_IMG, W_IMG)


def _forward_jax(x, Wqkv, bqkv, proj_w, proj_b, dwc_w, dwc_b,
                 an_bias, na_bias, pb2, ab2, mh, mw):
    import jax.numpy as jnp
    import jax
    b = x.shape[0]
    c, n, nh, A, hd = C, N, NUM_HEADS, AGENT_NUM, HD

    # bias tables built on-device from the tiny raw biases
    pb1 = jnp.einsum("Hj,hajk,Wk->haHW", mh, an_bias, mw).reshape(nh, A, n)
    pos_bias = pb1[None] + pb2                                  # (1, H, A, n)
    ab1 = jnp.einsum("Hj,hajk,Wk->haHW", mh, na_bias, mw).reshape(nh, A, n)
    agent_bias = ab1[None].transpose(0, 1, 3, 2) + ab2          # (1, H, n, A)

    xf = x.reshape(b, c, n).transpose(0, 2, 1)
    qkv = xf @ Wqkv + bqkv
    q, k, v = qkv[..., :c], qkv[..., c:2 * c], qkv[..., 2 * c:]

    qi = q.reshape(b, POOL, H_IMG // POOL, POOL, W_IMG // POOL, c)
    agent = qi.mean(axis=(2, 4)).reshape(b, A, c)

    qh = q.reshape(b, n, nh, hd).transpose(0, 2, 1, 3)
    kh = k.reshape(b, n, nh, hd).transpose(0, 2, 1, 3)
    vh = v.reshape(b, n, nh, hd).transpose(0, 2, 1, 3)
    ah = agent.reshape(b, A, nh, hd).transpose(0, 2, 1, 3)

    attn1 = jax.nn.softmax(
        jnp.einsum("bhad,bhnd->bhan", ah * SCALE, kh) + pos_bias, axis=-1)
    agent_v = jnp.einsum("bhan,bhnd->bhad", attn1, vh)

    attn2 = jax.nn.softmax(
        jnp.einsum("bhnd,bhad->bhna", qh * SCALE, ah) + agent_bias, axis=-1)
    out = jnp.einsum("bhna,bhad->bhnd", attn2, agent_v)
    out = out.transpose(0, 2, 1, 3).reshape(b, n, c)

    # depthwise 3x3 via 9 shifted adds (avoids conv lowering issues)
    vimg = vh.transpose(0, 2, 1, 3).reshape(b, H_IMG, W_IMG, c).transpose(0, 3, 1, 2)
    vp = jnp.pad(vimg, ((0, 0), (0, 0), (1, 1), (1, 1)))
    dw = jnp.zeros_like(vimg)
    for di in range(3):
        for dj in range(3):
            dw = dw + dwc_w[None, :, 0, di, dj, None, None] * \
                jax.lax.dynamic_slice(vp, (0, 0, di, dj), (b, c, H_IMG, W_IMG))
    dw = dw + dwc_b[None, :, None, None]
    out = out + dw.transpose(0, 2, 3, 1).reshape(b, n, c)

    out = out @ proj_w + proj_b
    return out.transpose(0, 2, 1).reshape(b, c, H_IMG, W_IMG)


_PMAPPED = None


def _get_pmapped():
    global _PMAPPED
    if _PMAPPED is None:
        import jax
        devs = jax.devices()[:N_CORES]
        _PMAPPED = jax.pmap(
            _forward_jax,
            devices=devs,
            in_axes=(0,) + (None,) * 12,
        )
    return _PMAPPED


def kernel(x, Wqkv, bqkv, proj_w, proj_b, dwc_w, dwc_b,
           an_bias, na_bias, ah_bias, aw_bias, ha_bias, wa_bias):
    x = np.ascontiguousarray(np.asarray(x, dtype=np.float32))
    Wqkv = np.asarray(Wqkv, np.float32)
    bqkv = np.asarray(bqkv, np.float32)
    proj_w = np.asarray(proj_w, np.float32)
    proj_b = np.asarray(proj_b, np.float32)
    dwc_w = np.asarray(dwc_w, np.float32)
    dwc_b = np.asarray(dwc_b, np.float32)
    an_bias = np.asarray(an_bias, np.float32)
    na_bias = np.asarray(na_bias, np.float32)
    ah_bias = np.asarray(ah_bias, np.float32)
    aw_bias = np.asarray(aw_bias, np.float32)
    ha_bias = np.asarray(ha_bias, np.float32)
    wa_bias = np.asarray(wa_bias, np.float32)

    pb2 = (ah_bias + aw_bias).reshape(1, NUM_HEADS, AGENT_NUM, N).astype(np.float32)
    ab2 = (ha_bias + wa_bias).reshape(1, NUM_HEADS, N, AGENT_NUM).astype(np.float32)
    mh = _interp_matrix(H_IMG, POOL)
    mw = _interp_matrix(W_IMG, POOL)

    try:
        fn = _get_pmapped()
        xs = x.reshape(N_CORES, B // N_CORES, C, H_IMG, W_IMG)
        out = fn(xs, Wqkv, bqkv, proj_w, proj_b, dwc_w, dwc_b,
                 an_bias, na_bias, pb2, ab2, mh, mw)
        out = np.asarray(out).reshape(B, C, H_IMG, W_IMG)
        if not np.all(np.isfinite(out)):
            raise RuntimeError("non-finite output from device path")
        return out.astype(np.float32)
    except Exception:
        pos_bias, agent_bias = _np_pos_biases(
            an_bias, na_bias, ah_bias, aw_bias, ha_bias, wa_bias)
        return _forward_np(x, Wqkv, bqkv, proj_w, proj_b, dwc_w, dwc_b,
                           pos_bias, agent_bias).astype(np.float32)



# revision 2
# speedup vs baseline: 4.0314x; 4.0314x over previous
import hashlib
import threading

import numpy as np

B, C, H_IMG, W_IMG = 32, 192, 56, 56
NUM_HEADS, AGENT_NUM, POOL = 6, 49, 7
N_CORES = 8
N = H_IMG * W_IMG
HD = C // NUM_HEADS
SCALE = HD ** (-0.5)
BS = B // N_CORES  # per-device batch


def _interp_matrix(out_size: int, in_size: int) -> np.ndarray:
    """Bilinear (half-pixel-center, edge-clamped) interpolation matrix.

    Matches jax.image.resize(method="linear") for upsampling: row o gives the
    weights over input cells for output cell o.
    """
    m = np.zeros((out_size, in_size), dtype=np.float64)
    ratio = in_size / out_size
    for o in range(out_size):
        s = (o + 0.5) * ratio - 0.5
        i0 = int(np.floor(s))
        frac = s - i0
        lo = min(max(i0, 0), in_size - 1)
        hi = min(max(i0 + 1, 0), in_size - 1)
        m[o, lo] += 1.0 - frac
        m[o, hi] += frac
    return m.astype(np.float32)


# ---------------------------------------------------------------- device path

_LOCK = threading.Lock()
_DEV = None  # dict: devices, jitted fns
_CONSTS = None  # (digest, [per-device tuple of device arrays])


def _fwd(x16, Wqkv, bqkv, proj_w, proj_b, dwc_w, dwc_b, pos_bias, agent_bias):
    import jax
    import jax.numpy as jnp

    b, c, n, nh, A, hd = BS, C, N, NUM_HEADS, AGENT_NUM, HD

    x = x16.astype(jnp.float32)                          # (b, c, h, w)
    xf = x.reshape(b, c, n).transpose(0, 2, 1)           # (b, n, c)
    qkv = xf @ Wqkv + bqkv
    q, k, v = qkv[..., :c], qkv[..., c:2 * c], qkv[..., 2 * c:]

    qi = q.reshape(b, POOL, H_IMG // POOL, POOL, W_IMG // POOL, c)
    agent = qi.mean(axis=(2, 4)).reshape(b, A, c)

    qh = q.reshape(b, n, nh, hd).transpose(0, 2, 1, 3)   # (b, H, n, d)
    kh = k.reshape(b, n, nh, hd).transpose(0, 2, 1, 3)
    vh = v.reshape(b, n, nh, hd).transpose(0, 2, 1, 3)
    ah = agent.reshape(b, A, nh, hd).transpose(0, 2, 1, 3)

    attn1 = jax.nn.softmax(
        jnp.einsum("bhad,bhnd->bhan", ah * SCALE, kh) + pos_bias, axis=-1)
    agent_v = jnp.einsum("bhan,bhnd->bhad", attn1, vh)   # (b, H, A, d)

    attn2 = jax.nn.softmax(
        jnp.einsum("bhnd,bhad->bhna", qh * SCALE, ah) + agent_bias, axis=-1)
    out = jnp.einsum("bhna,bhad->bhnd", attn2, agent_v)  # (b, H, n, d)
    out = out.transpose(0, 2, 1, 3).reshape(b, n, c)

    # depthwise 3x3 via 9 shifted adds
    vimg = vh.transpose(0, 2, 1, 3).reshape(b, H_IMG, W_IMG, c).transpose(0, 3, 1, 2)
    vp = jnp.pad(vimg, ((0, 0), (0, 0), (1, 1), (1, 1)))
    dw = jnp.zeros_like(vimg)
    for di in range(3):
        for dj in range(3):
            dw = dw + dwc_w[None, :, 0, di, dj, None, None] * \
                jax.lax.dynamic_slice(vp, (0, 0, di, dj), (b, c, H_IMG, W_IMG))
    dw = dw + dwc_b[None, :, None, None]
    out = out + dw.transpose(0, 2, 3, 1).reshape(b, n, c)

    out = out @ proj_w + proj_b
    return out.transpose(0, 2, 1).reshape(b, c, H_IMG, W_IMG).astype(jnp.float16)


def _prep(an_bias, na_bias, pb2, ab2, mh, mw):
    """Build the two dense bias tables on-device from the tiny raw biases."""
    import jax.numpy as jnp

    nh, A, n = NUM_HEADS, AGENT_NUM, N
    pb1 = jnp.einsum("Hj,hajk,Wk->haHW", mh, an_bias, mw).reshape(nh, A, n)
    pos_bias = pb1[None] + pb2                                  # (1, H, A, n)
    ab1 = jnp.einsum("Hj,hajk,Wk->haHW", mh, na_bias, mw).reshape(nh, A, n)
    agent_bias = ab1[None].transpose(0, 1, 3, 2) + ab2          # (1, H, n, A)
    return pos_bias, agent_bias


def _get_dev():
    global _DEV
    if _DEV is None:
        import jax
        devs = jax.devices()[:N_CORES]
        _DEV = {
            "devs": devs,
            "fwd": jax.jit(_fwd),
            "prep": jax.jit(_prep),
        }
    return _DEV


def _get_consts(Wqkv, bqkv, proj_w, proj_b, dwc_w, dwc_b,
                an_bias, na_bias, ah_bias, aw_bias, ha_bias, wa_bias):
    """Per-device tuples of device-resident constants, cached across calls."""
    global _CONSTS
    hsh = hashlib.blake2b(digest_size=16)
    for a in (Wqkv, bqkv, proj_w, proj_b, dwc_w, dwc_b,
              an_bias, na_bias, ah_bias, aw_bias, ha_bias, wa_bias):
        hsh.update(np.ascontiguousarray(a).tobytes())
    digest = hsh.digest()
    if _CONSTS is not None and _CONSTS[0] == digest:
        return _CONSTS[1]

    import jax
    d = _get_dev()
    pb2 = (ah_bias + aw_bias).reshape(1, NUM_HEADS, AGENT_NUM, N).astype(np.float32)
    ab2 = (ha_bias + wa_bias).reshape(1, NUM_HEADS, N, AGENT_NUM).astype(np.float32)
    mh = _interp_matrix(H_IMG, POOL)
    mw = _interp_matrix(W_IMG, POOL)

    per_dev = []
    for dev in d["devs"]:
        put = lambda a: jax.device_put(np.asarray(a, np.float32), dev)
        pos_bias, agent_bias = d["prep"](
            put(an_bias), put(na_bias), put(pb2), put(ab2), put(mh), put(mw))
        per_dev.append((put(Wqkv), put(bqkv), put(proj_w), put(proj_b),
                        put(dwc_w), put(dwc_b), pos_bias, agent_bias))
    for t in per_dev:
        for a in t:
            a.block_until_ready()
    _CONSTS = (digest, per_dev)
    return per_dev


def _run_device(x, consts):
    """Shard over 8 devices; fp16 both ways; threads so up/down overlap."""
    import jax
    d = _get_dev()
    devs, fwd = d["devs"], d["fwd"]

    outs = [None] * N_CORES
    errs = []

    def work(j):
        try:
            xj = x[j * BS:(j + 1) * BS].astype(np.float16)
            xd = jax.device_put(xj, devs[j])
            od = fwd(xd, *consts[j])
            outs[j] = np.asarray(od)
        except Exception as e:  # noqa: BLE001
            errs.append(e)

    threads = [threading.Thread(target=work, args=(j,)) for j in range(N_CORES)]
    for t in threads:
        t.start()
    for t in threads:
        t.join()
    if errs:
        raise errs[0]
    out = np.concatenate(outs, axis=0)
    return np.ascontiguousarray(out.astype(np.float32))


# ---------------------------------------------------------------- numpy fallback

def _np_pos_biases(an_bias, na_bias, ah_bias, aw_bias, ha_bias, wa_bias):
    mh = _interp_matrix(H_IMG, POOL)
    mw = _interp_matrix(W_IMG, POOL)
    pb1 = np.einsum("Hj,hajk,Wk->haHW", mh, an_bias, mw).reshape(NUM_HEADS, AGENT_NUM, N)
    pos_bias = (pb1[None] + (ah_bias + aw_bias).reshape(1, NUM_HEADS, AGENT_NUM, N))
    ab1 = np.einsum("Hj,hajk,Wk->haHW", mh, na_bias, mw).reshape(NUM_HEADS, AGENT_NUM, N)
    agent_bias = (ab1[None].transpose(0, 1, 3, 2)
                  + (ha_bias + wa_bias).reshape(1, NUM_HEADS, N, AGENT_NUM))
    return pos_bias.astype(np.float32), agent_bias.astype(np.float32)


def _forward_np(x, Wqkv, bqkv, proj_w, proj_b, dwc_w, dwc_b,
                pos_bias, agent_bias):
    b = x.shape[0]
    c, n, nh, A, hd = C, N, NUM_HEADS, AGENT_NUM, HD

    xf = x.reshape(b, c, n).transpose(0, 2, 1)
    qkv = xf @ Wqkv + bqkv
    q, k, v = qkv[..., :c], qkv[..., c:2 * c], qkv[..., 2 * c:]

    qi = q.reshape(b, POOL, H_IMG // POOL, POOL, W_IMG // POOL, c)
    agent = qi.mean(axis=(2, 4)).reshape(b, A, c)

    qh = q.reshape(b, n, nh, hd).transpose(0, 2, 1, 3)
    kh = k.reshape(b, n, nh, hd).transpose(0, 2, 1, 3)
    vh = v.reshape(b, n, nh, hd).transpose(0, 2, 1, 3)
    ah = agent.reshape(b, A, nh, hd).transpose(0, 2, 1, 3)

    s1 = np.einsum("bhad,bhnd->bhan", ah * SCALE, kh) + pos_bias
    s1 = s1 - s1.max(axis=-1, keepdims=True)
    e1 = np.exp(s1)
    attn1 = e1 / e1.sum(axis=-1, keepdims=True)
    agent_v = np.einsum("bhan,bhnd->bhad", attn1, vh)

    s2 = np.einsum("bhnd,bhad->bhna", qh * SCALE, ah) + agent_bias
    s2 = s2 - s2.max(axis=-1, keepdims=True)
    e2 = np.exp(s2)
    attn2 = e2 / e2.sum(axis=-1, keepdims=True)
    out = np.einsum("bhna,bhad->bhnd", attn2, agent_v)
    out = out.transpose(0, 2, 1, 3).reshape(b, n, c)

    vimg = vh.transpose(0, 2, 1, 3).reshape(b, H_IMG, W_IMG, c).transpose(0, 3, 1, 2)
    vp = np.pad(vimg, ((0, 0), (0, 0), (1, 1), (1, 1)))
    dw = np.zeros_like(vimg)
    for di in range(3):
        for dj in range(3):
            dw += dwc_w[None, :, 0, di, dj, None, None] * \
                vp[:, :, di:di + H_IMG, dj:dj + W_IMG]
    dw = dw + dwc_b[None, :, None, None]
    out = out + dw.transpose(0, 2, 3, 1).reshape(b, n, c)

    out = out @ proj_w + proj_b
    return out.transpose(0, 2, 1).reshape(b, c, H_IMG, W_IMG)


# ---------------------------------------------------------------- entry point

def kernel(x, Wqkv, bqkv, proj_w, proj_b, dwc_w, dwc_b,
           an_bias, na_bias, ah_bias, aw_bias, ha_bias, wa_bias):
    x = np.ascontiguousarray(np.asarray(x, dtype=np.float32))
    Wqkv = np.asarray(Wqkv, np.float32)
    bqkv = np.asarray(bqkv, np.float32)
    proj_w = np.asarray(proj_w, np.float32)
    proj_b = np.asarray(proj_b, np.float32)
    dwc_w = np.asarray(dwc_w, np.float32)
    dwc_b = np.asarray(dwc_b, np.float32)
    an_bias = np.asarray(an_bias, np.float32)
    na_bias = np.asarray(na_bias, np.float32)
    ah_bias = np.asarray(ah_bias, np.float32)
    aw_bias = np.asarray(aw_bias, np.float32)
    ha_bias = np.asarray(ha_bias, np.float32)
    wa_bias = np.asarray(wa_bias, np.float32)

    try:
        with _LOCK:
            consts = _get_consts(Wqkv, bqkv, proj_w, proj_b, dwc_w, dwc_b,
                                 an_bias, na_bias, ah_bias, aw_bias,
                                 ha_bias, wa_bias)
            out = _run_device(x, consts)
        if not np.all(np.isfinite(out)):
            raise RuntimeError("non-finite output from device path")
        return out
    except Exception:
        pos_bias, agent_bias = _np_pos_biases(
            an_bias, na_bias, ah_bias, aw_bias, ha_bias, wa_bias)
        return _forward_np(x, Wqkv, bqkv, proj_w, proj_b, dwc_w, dwc_b,
                           pos_bias, agent_bias).astype(np.float32)
